# revision 25
# baseline (speedup 1.0000x reference)
"""DeeperGCN (GENConv softmax-aggr, 12 layers) on 8 Trainium2 NeuronCores.

Strategy: nodes are sharded across 8 cores by dst; each core owns its nodes'
incoming edges, packed dst-sorted into 128-edge chunks aligned to fixed
32-node PSUM windows. Per layer: all-gather bf16 node table -> dma_gather
src features + CCE-accumulate edge encodings -> relu/exp/mul ->
segment-softmax sums via TensorE matmuls (S.T @ [w|q]) into PSUM ->
nodewise MLP/LN in bf16 with fp32 residual state.

Softmax restructure (exact up to fp assoc):
  m = relu(y)+eps, logits = t*m; with shift-invariance and t>0:
  w_e = exp(t*relu(y)) ; den = sum w_e ; num = sum relu(y)*w_e
  agg = num/den + eps   (eps term exact to ~1e-16; empty segs give 0)
"""
import sys
import os
import numpy as np

if "/opt/trn_rl_repo" not in sys.path:
    sys.path.insert(0, "/opt/trn_rl_repo")

import ml_dtypes

# ---------------- problem constants (hardcoded per spec) ----------------
N = 20000
E = 640000
H = 128
L = 12
DIN = 3
DE = 4
DOUT = 16
EPS = 1e-7
LN_EPS = 1e-5
NCORES = 8
NLOC = N // NCORES          # 2500 nodes per core
BLK = 128                   # nodes per block (psum tile columns / partitions)
NBLK = (NLOC + BLK - 1) // BLK   # 20 blocks (last one partial)
NSLOT = NBLK * BLK          # 2560 node slots per core
WS = 32                     # psum window width (nodes per window)
NWIN = BLK // WS            # 4 windows per block
BPG = 1                     # blocks per dma_gather call

BF16 = ml_dtypes.bfloat16

# all-gather block groups (uneven: keep the final, critical-path one small)
GRP_NBLK = [6, 6, 6, 2]
GRP_BASE_BLK = [0, 6, 12, 18]
GRP_OF_BLK = [0] * 6 + [1] * 6 + [2] * 6 + [3] * 2
# trigger group g after this block's nodewise (g2/g3 fire after the loop)
GRP_TRIG_BLK = {9: 0, 15: 1}


# ======================================================================
# Host-side prep: shard + sort + pack the graph into per-core tensors
# ======================================================================
def _prep_graph(edge_index, edge_attr):
    src = np.asarray(edge_index[0]).astype(np.int64)
    dst = np.asarray(edge_index[1]).astype(np.int64)
    e_attr = np.asarray(edge_attr, dtype=np.float32)

    deg = np.bincount(dst, minlength=N)

    # --- node -> (core, block, col) slot assignment, degree balanced ---
    # LPT over blocks: equalizes per-block edge counts so the global max
    # window load (and thus c_win / e_pad) is minimal. Then LPT over the
    # 4 windows within each block.
    col_of = np.zeros(N, np.int32)
    blk_of = np.zeros(N, np.int32)
    for r in range(NCORES):
        nodes = np.arange(r * NLOC, (r + 1) * NLOC)
        order = nodes[np.argsort(-deg[nodes], kind="stable")]
        bload = np.zeros(NBLK, np.int64)
        bcnt = np.zeros(NBLK, np.int64)
        for nd in order:
            bsel = min(
                (bb for bb in range(NBLK) if bcnt[bb] < BLK),
                key=lambda bb: bload[bb],
            )
            blk_of[nd] = bsel
            bcnt[bsel] += 1
            bload[bsel] += deg[nd]
        for b in range(NBLK):
            bn = nodes[blk_of[nodes] == b]
            order = bn[np.argsort(-deg[bn], kind="stable")]
            wload = np.zeros(NWIN, np.int64)
            wcnt = np.zeros(NWIN, np.int64)
            for nd in order:
                j = min(
                    (jj for jj in range(NWIN) if wcnt[jj] < WS),
                    key=lambda jj: wload[jj],
                )
                col_of[nd] = j * WS + wcnt[j]
                wcnt[j] += 1
                wload[j] += deg[nd]
    # global table slot of each node (row in the all-gathered hn table)
    slot_of = (np.arange(N) // NLOC) * NSLOT + blk_of * BLK + col_of
    # table row under the chunked all-gather layout:
    # uneven groups [6,6,6,2]: the last collective (the only one on the
    # layer-boundary critical path) stays small
    own_of = np.arange(N) // NLOC
    grp_of = np.asarray(GRP_OF_BLK, np.int64)[blk_of]
    gbase = np.asarray(GRP_BASE_BLK, np.int64)[grp_of]
    gnblk = np.asarray(GRP_NBLK, np.int64)[grp_of]
    tslot_of = (gbase * (NCORES * BLK) + own_of * (gnblk * BLK)
                + (blk_of - gbase) * BLK + col_of)

    # --- chunk schedule: C chunks per block, round-robin over windows ---
    # window j of block b needs ceil(D_bj/128) chunks; C_WIN = global max
    own = dst // NLOC
    win_of_dst = col_of[dst] // WS
    blk_of_dst = blk_of[dst]
    # count edges per (core, block, window)
    key = (own * NBLK + blk_of_dst) * NWIN + win_of_dst
    wdeg = np.bincount(key, minlength=NCORES * NBLK * NWIN)
    c_win = int(np.max((wdeg + 127) // 128))
    C = NWIN * c_win
    e_pad = NBLK * C * 128
    assert e_pad % 16 == 0

    # sort edges by (core, block, window, col) so each window's edges are
    # contiguous and dst-col sorted
    sortkey = (key * BLK + col_of[dst]) * 1  # (core,blk,win,col)
    order = np.argsort(sortkey, kind="stable")
    s_src, s_dst = src[order], dst[order]
    s_attr = e_attr[order]

    gidx = np.zeros((NCORES, e_pad), np.int32)            # gather table rows
    attr_p = np.zeros((NCORES, e_pad, DE), np.float32)    # padded edge attrs
    scol = np.full((NCORES, e_pad), -1, np.int32)         # dst col in window (for S)

    # per (core, block, window): window j's chunks are consecutive
    # (c = j*c_win + q) so each window's psum group is sequential
    start = np.zeros(NCORES * NBLK * NWIN + 1, np.int64)
    np.cumsum(np.bincount(key[order], minlength=NCORES * NBLK * NWIN), out=start[1:])
    for r in range(NCORES):
        for b in range(NBLK):
            for j in range(NWIN):
                k = (r * NBLK + b) * NWIN + j
                e0, e1 = start[k], start[k + 1]
                nloc_e = e1 - e0
                # edge i of this window -> chunk q=i//128, slot s=i%128
                q, s = np.arange(nloc_e) // 128, np.arange(nloc_e) % 128
                epos = (b * C + j * c_win + q) * 128 + s
                gidx[r, epos] = tslot_of[s_src[e0:e1]]
                attr_p[r, epos] = s_attr[e0:e1]
                scol[r, epos] = col_of[s_dst[e0:e1]] - j * WS

    # --- build per-core device arrays ---
    # S: [128 slots, NBLK*C chunks, WS] one-hot dst-col matrices
    nchunks = NBLK * C
    S = np.zeros((NCORES, 128, nchunks, WS), np.float32)
    cc = (np.arange(e_pad) // 128)
    ss = (np.arange(e_pad) % 128)
    for r in range(NCORES):
        valid = scol[r] >= 0
        S[r, ss[valid], cc[valid], scol[r][valid]] = 1.0

    # idxs int16 [128, e_pad//16]: idx i at partition i%16, col i//16, x8
    idxs = np.zeros((NCORES, 128, e_pad // 16), np.int16)
    for r in range(NCORES):
        w = gidx[r].reshape(e_pad // 16, 16).T.astype(np.int16)  # [16, cols]
        idxs[r] = np.tile(w, (8, 1))

    # attr transposed [4, e_pad]
    attr_t = np.ascontiguousarray(attr_p.transpose(0, 2, 1))

    return dict(C=C, e_pad=e_pad, idxs=idxs, attr_t=attr_t,
                S=S.astype(BF16), slot_of=slot_of, col_of=col_of)


def _prep_inputs(inputs):
    g = _prep_graph(inputs["edge_index"], inputs["edge_attr"])
    x = np.asarray(inputs["x"], np.float32)
    # x_t [3, NSLOT] per core, node at its slot
    x_t = np.zeros((NCORES, DIN, NSLOT), np.float32)
    slot = g["slot_of"]
    for r in range(NCORES):
        nodes = np.arange(r * NLOC, (r + 1) * NLOC)
        x_t[r][:, slot[nodes] - r * NSLOT] = x[nodes].T

    f32 = np.float32
    w = {k: np.asarray(v, f32) for k, v in inputs.items()
         if k not in ("x", "edge_index", "edge_attr")}
    # triviality flags (structurally ones/zeros in setup_inputs)
    triv = (np.all(w["enc_node_b"] == 0) and np.all(w["enc_edge_b"] == 0)
            and np.all(w["mlp1_b"] == 0) and np.all(w["mlp2_b"] == 0)
            and np.all(w["mlp_ln_g"] == 1) and np.all(w["mlp_ln_b"] == 0)
            and np.all(w["ln_g"] == 1) and np.all(w["ln_b"] == 0)
            and np.all(w["lin_b"] == 0))
    assert triv, "non-trivial bias/LN params not supported by this kernel build"
    t_vals = [float(v) for v in np.asarray(inputs["t"], f32)]
    assert all(tv > 0 for tv in t_vals)

    w1_all = np.zeros((128, L * 2 * H), BF16)
    w2_all = np.zeros((128, L * 2 * H), BF16)
    for l in range(L):
        w1_all[:, l * 256:(l + 1) * 256] = w["mlp1_w"][l].astype(BF16)
        w2_all[:, (2 * l) * 128:(2 * l + 1) * 128] = w["mlp2_w"][l][0:128].astype(BF16)
        w2_all[:, (2 * l + 1) * 128:(2 * l + 2) * 128] = w["mlp2_w"][l][128:256].astype(BF16)

    per_core_common = dict(
        enc_node_w=np.asarray(w["enc_node_w"], f32),
        enc_edge_w=np.asarray(w["enc_edge_w"], f32),
        w1_all=w1_all, w2_all=w2_all,
        lin_w=np.asarray(w["lin_w"], f32).astype(BF16),
        ident=np.eye(128, dtype=BF16),
    )
    in_maps = []
    for r in range(NCORES):
        m = dict(per_core_common)
        m["x_t"] = x_t[r]
        m["attr_t"] = g["attr_t"][r]
        m["idxs_in"] = g["idxs"][r]
        m["s_in"] = g["S"][r]
        in_maps.append(m)
    return g, in_maps, t_vals


# ======================================================================
# Device program
# ======================================================================
DEBUG_TAPS = False


def _build_program(C, t_vals):
    import concourse.bass as bass
    import concourse.bacc as bacc
    import concourse.tile as tile
    import concourse.mybir as mybir

    dt = mybir.dt
    AF = mybir.ActivationFunctionType
    ALU = mybir.AluOpType
    c_win = C // NWIN
    e_pad = NBLK * C * 128
    nchunks = NBLK * C

    nc = bacc.Bacc("TRN2", target_bir_lowering=False, debug=False,
                   num_devices=NCORES, num_swdge_queues=4)

    def din(name, shape, dty):
        return nc.dram_tensor(name, shape, dty, kind="ExternalInput").ap()

    x_t = din("x_t", [DIN, NSLOT], dt.float32)
    attr_t = din("attr_t", [DE, e_pad], dt.float32)
    idxs_in = din("idxs_in", [128, e_pad // 16], dt.int16)
    s_in = din("s_in", [128, nchunks, WS], dt.bfloat16)
    enc_node_w = din("enc_node_w", [DIN, H], dt.float32)
    enc_edge_w = din("enc_edge_w", [DE, H], dt.float32)
    w1_in = din("w1_all", [128, L * 256], dt.bfloat16)
    w2_in = din("w2_all", [128, L * 256], dt.bfloat16)
    lin_in = din("lin_w", [128, DOUT], dt.bfloat16)
    ident_in = din("ident", [128, 128], dt.bfloat16)
    out_t = nc.dram_tensor("out_t", [DOUT, NSLOT], dt.float32,
                           kind="ExternalOutput").ap()
    dbg = {}
    if DEBUG_TAPS:
        e_pad_ = NBLK * C * 128
        for nm, shp, dty in [
            ("dbg_h0", [128, NSLOT], dt.float32),
            ("dbg_ea", [128, 1024], dt.bfloat16),
            ("dbg_tab", [NCORES * NSLOT, H], dt.bfloat16),
            ("dbg_y", [128, C * 128], dt.bfloat16),
            ("dbg_r", [128, C * 128], dt.bfloat16),
            ("dbg_wq", [128, C * 256], dt.bfloat16),
            ("dbg_dn", [128, 256], dt.float32),
            ("dbg_z", [128, H], dt.bfloat16),
            ("dbg_u", [128, 256], dt.bfloat16),
            ("dbg_h1", [128, NSLOT], dt.float32),
        ]:
            dbg[nm] = nc.dram_tensor(nm, shp, dty, kind="ExternalOutput").ap()

    # persistent SBUF
    sb = lambda name, shape, dty: nc.alloc_sbuf_tensor(name, list(shape), dty).ap()
    x_sb = sb("x_sb", [DIN, NSLOT], dt.float32)
    idxs_sb = sb("idxs_sb", [128, e_pad // 16], dt.int16)
    s_sb = sb("s_sb", [128, nchunks, WS], dt.bfloat16)
    encn_sb = sb("encn_sb", [DIN, H], dt.float32)
    ence_sb = sb("ence_sb", [DE, H], dt.float32)
    w1_sb = sb("w1_sb", [128, L * 256], dt.bfloat16)
    w2_sb = sb("w2_sb", [128, L * 256], dt.bfloat16)
    lin_sb = sb("lin_sb", [128, DOUT], dt.bfloat16)
    ident_sb = sb("ident_sb", [128, 128], dt.bfloat16)
    h_sb = sb("h_sb", [128, NSLOT], dt.float32)      # [node_in_blk, blk*H+ch] fp32 state
    eps_sb = sb("eps_sb", [128, 1], dt.float32)      # LN_EPS per-partition const
    hn_sb = sb("hn_sb", [128, NSLOT], dt.bfloat16)   # relu(LN(h)) per block
    zero_sb = sb("zero_sb", [128, C * 128], dt.bfloat16)  # relu via tt-max

    RG = [list(range(NCORES))]

    with tile.TileContext(nc) as tc:
        with (
            tc.tile_pool(name="dram", bufs=1, space="DRAM") as dram,
            tc.tile_pool(name="ypool", bufs=4) as ypool,
            tc.tile_pool(name="eapool", bufs=3) as eapool,
            tc.tile_pool(name="wqpool", bufs=2) as wqpool,
            tc.tile_pool(name="nsmall", bufs=3) as nsmall,
            tc.tile_pool(name="nwork", bufs=3) as nwork,
            tc.tile_pool(name="dn_ps", bufs=2, space="PSUM") as dn_ps_pool,
            tc.tile_pool(name="mlp_ps", bufs=2, space="PSUM") as mlp_ps_pool,
            tc.tile_pool(name="tp_ps", bufs=2, space="PSUM") as tp_ps_pool,
            tc.tile_pool(name="setup", bufs=3) as setup_pool,
        ):
            ea_d = dram.tile([128, e_pad], dt.bfloat16)
            ag_in = dram.tile([NSLOT, H], dt.bfloat16)
            # double-buffered node table: layer l gathers from tab[l%2],
            # its collectives write tab[(l+1)%2] (no WAR with own gathers)
            tab = [dram.tile([NCORES * NSLOT, H], dt.bfloat16, name=f"tab{i}")
                   for i in range(2)]

            # ---------------- load constants ----------------
            nc.sync.dma_start(x_sb, x_t)
            nc.sync.dma_start(idxs_sb, idxs_in)
            nc.sync.dma_start(s_sb, s_in)
            nc.sync.dma_start(encn_sb, enc_node_w)
            nc.sync.dma_start(ence_sb, enc_edge_w)
            nc.sync.dma_start(w1_sb, w1_in)
            nc.sync.dma_start(w2_sb, w2_in)
            nc.sync.dma_start(lin_sb, lin_in)
            nc.sync.dma_start(ident_sb, ident_in)
            nc.gpsimd.memset(eps_sb, LN_EPS)
            nc.gpsimd.memset(zero_sb, 0.0)

            # ---------------- encoders ----------------
            for b in range(NBLK):
                h_ps = tp_ps_pool.tile([128, H], dt.float32, tag="tp")
                nc.tensor.matmul(h_ps, lhsT=x_sb[:, b * 128:(b + 1) * 128],
                                 rhs=encn_sb, start=True, stop=True)
                nc.vector.tensor_copy(h_sb[:, b * H:(b + 1) * H], h_ps)
                nc.scalar.copy(hn_sb[:, b * H:(b + 1) * H], h_ps)
                nc.sync.dma_start(ag_in[b * 128:(b + 1) * 128, :],
                                  hn_sb[:, b * H:(b + 1) * H])
            for g4 in range(nchunks // 4):
                atile = setup_pool.tile([DE, 512], dt.float32, tag="attr")
                nc.sync.dma_start(atile, attr_t[:, g4 * 512:(g4 + 1) * 512])
                ea_ps = mlp_ps_pool.tile([128, 512], dt.float32, tag="mlp")
                for k in range(4):
                    nc.tensor.matmul(ea_ps[:, k * 128:(k + 1) * 128],
                                     lhsT=atile[:, k * 128:(k + 1) * 128],
                                     rhs=ence_sb, start=True, stop=True)
                ea_sb = setup_pool.tile([128, 512], dt.bfloat16, tag="east")
                if g4 % 2 == 0:
                    nc.vector.tensor_copy(ea_sb, ea_ps)
                else:
                    nc.scalar.copy(ea_sb, ea_ps)
                nc.sync.dma_start(ea_d[:, g4 * 512:(g4 + 1) * 512], ea_sb)
            def chunk_ag(g4c, dst_tab):
                a0 = GRP_BASE_BLK[g4c] * 128
                a1 = a0 + GRP_NBLK[g4c] * 128
                t0 = GRP_BASE_BLK[g4c] * NCORES * 128
                t1 = t0 + GRP_NBLK[g4c] * NCORES * 128
                nc.gpsimd.collective_compute(
                    "AllGather", ALU.bypass, replica_groups=RG,
                    ins=[ag_in[a0:a1, :]], outs=[dst_tab[t0:t1, :]])

            for g4c in range(4):
                chunk_ag(g4c, tab[0])
            if DEBUG_TAPS:
                nc.sync.dma_start(dbg["dbg_h0"], h_sb)
                nc.sync.dma_start(dbg["dbg_ea"], ea_d[:, 0:1024])
                nc.sync.dma_start(dbg["dbg_tab"], tab[0][:, :])

            # ---------------- layers ----------------
            for l in range(L):
                y = None
                for b in range(NBLK):
                    ni = C * 128
                    y = ypool.tile([128, C, 128], dt.bfloat16, tag="y")
                    half = (C * 128) // 2
                    for hf in range(2):
                        nc.gpsimd.dma_gather(
                            y[:, hf * (C // 2):(hf + 1) * (C // 2), :],
                            tab[l % 2][:, :],
                            idxs_sb[:, (b * C * 128 + hf * half) // 16:
                                    (b * C * 128 + (hf + 1) * half) // 16],
                            num_idxs=half, num_idxs_reg=half, elem_size=H,
                            single_packet=False, queue_num=(2 * b + hf) % 4)
                    eat = eapool.tile([128, C * 128], dt.bfloat16, tag="ea")
                    nc.sync.dma_start(eat, ea_d[:, b * C * 128:(b + 1) * C * 128])
                    yb = y[:, :, :]
                    # y += ea ; r = relu(y)  (both in place)
                    nc.vector.tensor_tensor(y.opt(), y.opt(), eat, ALU.add)
                    if DEBUG_TAPS and l == 0 and b == 0:
                        nc.sync.dma_start(dbg["dbg_y"], y.opt())
                    nc.vector.tensor_tensor(y.opt(), y.opt(), zero_sb, ALU.max)
                    if DEBUG_TAPS and l == 0 and b == 0:
                        nc.sync.dma_start(dbg["dbg_r"], y.opt())
                    wq = wqpool.tile([128, C, 256], dt.bfloat16, tag="wq")
                    nc.scalar.activation(wq[:, :, 0:128], yb, AF.Exp,
                                         scale=t_vals[l])
                    nc.vector.tensor_tensor(wq[:, :, 128:256], yb,
                                            wq[:, :, 0:128], ALU.mult)
                    if DEBUG_TAPS and l == 0 and b == 0:
                        nc.sync.dma_start(dbg["dbg_wq"], wq.opt())
                    dn = dn_ps_pool.tile([128, 256], dt.float32, tag="dn")
                    for c in range(C):
                        j, q = c // c_win, c % c_win
                        nc.tensor.matmul(
                            dn[32 * j:32 * j + 32, :],
                            lhsT=s_sb[:, b * C + c, :], rhs=wq[:, c, :],
                            start=(q == 0), stop=(q == c_win - 1),
                            tile_position=(0, 32 * j))
                    _nodewise(nc, tc, l, b, dn,
                              h_sb, hn_sb, w1_sb, w2_sb, lin_sb, ident_sb,
                              eps_sb, nsmall, nwork, mlp_ps_pool, tp_ps_pool,
                              ag_in, out_t, dt, AF, ALU, dbg)
                    # deferred chunk collectives: group g fires a few blocks
                    # after its last producer so the ag_in writes are done
                    # when the Pool queue reaches the trigger
                    if l < L - 1 and b in GRP_TRIG_BLK:
                        chunk_ag(GRP_TRIG_BLK[b], tab[(l + 1) % 2])
                if l < L - 1:
                    chunk_ag(2, tab[(l + 1) % 2])
                    chunk_ag(3, tab[(l + 1) % 2])

    nc.compile()
    return nc


def _nodewise(nc, tc, l, b, dn, h_sb, hn_sb, w1_sb, w2_sb, lin_sb, ident_sb,
              eps_sb, nsmall, nwork, mlp_ps_pool, tp_ps_pool, ag_in, out_t,
              dt, AF, ALU, dbg):
    H_ = H
    hsl = slice(b * H_, (b + 1) * H_)
    # agg = num/(den+1e-16);  z = agg + hn   [node, ch]
    tmp = nwork.tile([128, H_], dt.float32, tag="tmp")
    nc.vector.tensor_scalar_add(tmp, dn[:, 0:128], 1e-16)
    rec = nwork.tile([128, H_], dt.float32, tag="rec")
    nc.vector.reciprocal_approx_fast(rec, tmp)
    agg = nwork.tile([128, H_], dt.float32, tag="agg")
    nc.vector.tensor_tensor(agg, dn[:, 128:256], rec, ALU.mult)
    z = nwork.tile([128, H_], dt.bfloat16, tag="z")
    nc.vector.tensor_tensor(z, agg, hn_sb[:, hsl], ALU.add)
    if dbg and l == 0 and b == 0:
        dnc = nwork.tile([128, 256], dt.float32, tag="dbgdn")
        nc.vector.tensor_copy(dnc, dn)
        nc.sync.dma_start(dbg["dbg_dn"], dnc)
        nc.sync.dma_start(dbg["dbg_z"], z)
    # zT for mm1
    zT_ps = tp_ps_pool.tile([128, H_], dt.bfloat16, tag="tp")
    nc.tensor.transpose(zT_ps, z, ident_sb)
    zT = nwork.tile([128, H_], dt.bfloat16, tag="zT")
    nc.vector.tensor_copy(zT, zT_ps)
    # mm1: out1[n, 2H] = z @ w1
    out1 = mlp_ps_pool.tile([128, 256], dt.float32, tag="mlp")
    nc.tensor.matmul(out1, lhsT=zT, rhs=w1_sb[:, l * 256:(l + 1) * 256],
                     start=True, stop=True)
    # LN(out1) -> relu -> u  (g=1, b=0)
    st6 = nsmall.tile([128, 6], dt.float32, tag="st6")
    nc.vector.bn_stats(st6, out1)
    mv = nsmall.tile([128, 2], dt.float32, tag="mv")
    nc.vector.bn_aggr(mv, st6)
    sq = nsmall.tile([128, 1], dt.float32, tag="sq")
    nc.scalar.activation(sq, mv[:, 1:2], AF.Sqrt, bias=eps_sb[:, 0:1])
    rstd = nsmall.tile([128, 1], dt.float32, tag="rstd")
    nc.vector.reciprocal(rstd, sq)
    nmr = nsmall.tile([128, 1], dt.float32, tag="nmr")
    nc.vector.tensor_scalar(nmr, mv[:, 0:1], rstd[:, 0:1], -1.0,
                            ALU.mult, ALU.mult)
    u = nwork.tile([128, 256], dt.bfloat16, tag="u")
    nc.scalar.activation(u, out1, AF.Relu, bias=nmr[:, 0:1],
                         scale=rstd[:, 0:1])
    if dbg and l == 0 and b == 0:
        nc.sync.dma_start(dbg["dbg_u"], u)
    # uT (two halves) for mm2
    uT_ps = tp_ps_pool.tile([128, 256], dt.bfloat16, tag="tp")
    nc.tensor.transpose(uT_ps[:, 0:128], u[:, 0:128], ident_sb)
    nc.tensor.transpose(uT_ps[:, 128:256], u[:, 128:256], ident_sb)
    uT = nwork.tile([128, 256], dt.bfloat16, tag="uT")
    nc.vector.tensor_copy(uT, uT_ps)
    # mm2: dh[n, ch] = u @ w2
    out2 = mlp_ps_pool.tile([128, 128], dt.float32, tag="mlp")
    nc.tensor.matmul(out2, lhsT=uT[:, 0:128],
                     rhs=w2_sb[:, (2 * l) * 128:(2 * l + 1) * 128],
                     start=True, stop=False)
    nc.tensor.matmul(out2, lhsT=uT[:, 128:256],
                     rhs=w2_sb[:, (2 * l + 1) * 128:(2 * l + 2) * 128],
                     start=False, stop=True)
    # residual (layer 0: h = mlp out only)
    if l == 0:
        nc.vector.tensor_copy(h_sb[:, hsl], out2)
    else:
        nc.vector.tensor_tensor(h_sb[:, hsl], h_sb[:, hsl], out2, ALU.add)
    # prenorm -> hn (next layer's table; layer L-1 computes head input w/ g0)
    st6b = nsmall.tile([128, 6], dt.float32, tag="st6")
    nc.vector.bn_stats(st6b, h_sb[:, hsl])
    mvb = nsmall.tile([128, 2], dt.float32, tag="mv")
    nc.vector.bn_aggr(mvb, st6b)
    sqb = nsmall.tile([128, 1], dt.float32, tag="sq")
    nc.scalar.activation(sqb, mvb[:, 1:2], AF.Sqrt, bias=eps_sb[:, 0:1])
    rstdb = nsmall.tile([128, 1], dt.float32, tag="rstd")
    nc.vector.reciprocal(rstdb, sqb)
    nmrb = nsmall.tile([128, 1], dt.float32, tag="nmr")
    nc.vector.tensor_scalar(nmrb, mvb[:, 0:1], rstdb[:, 0:1], -1.0,
                            ALU.mult, ALU.mult)
    nc.scalar.activation(hn_sb[:, hsl], h_sb[:, hsl], AF.Relu,
                         bias=nmrb[:, 0:1], scale=rstdb[:, 0:1])
    if dbg and l == 0 and b == NBLK - 1:
        nc.sync.dma_start(dbg["dbg_h1"], h_sb)
    if l < L - 1:
        nc.sync.dma_start(ag_in[b * 128:(b + 1) * 128, :], hn_sb[:, hsl])
    else:
        # final head: out = relu(LN(h,g0)) @ lin_w
        hT_ps = tp_ps_pool.tile([128, H_], dt.bfloat16, tag="tp")
        nc.tensor.transpose(hT_ps, hn_sb[:, hsl], ident_sb)
        hT = nwork.tile([128, H_], dt.bfloat16, tag="zT")
        nc.vector.tensor_copy(hT, hT_ps)
        outf_ps = mlp_ps_pool.tile([DOUT, 128], dt.float32, tag="mlp")
        nc.tensor.matmul(outf_ps, lhsT=lin_sb, rhs=hT, start=True, stop=True)
        outf = nwork.tile([DOUT, 128], dt.float32, tag="outf")
        nc.vector.tensor_copy(outf, outf_ps)
        nc.sync.dma_start(out_t[:, b * 128:(b + 1) * 128], outf)


# ======================================================================
# Entry point
# ======================================================================
_CACHE = {}


def kernel(**inputs):
    g, in_maps, t_vals = _prep_inputs(inputs)
    C = g["C"]
    key = (C, tuple(t_vals))
    if key not in _CACHE:
        _CACHE[key] = _build_program(C, t_vals)
    nc = _CACHE[key]

    from concourse import bass_utils
    res = bass_utils.run_bass_kernel_spmd(
        nc, in_maps, core_ids=list(range(NCORES)))

    out = np.zeros((N, DOUT), np.float32)
    slot = g["slot_of"]
    for r in range(NCORES):
        nodes = np.arange(r * NLOC, (r + 1) * NLOC)
        out[nodes] = res.results[r]["out_t"][:, slot[nodes] - r * NSLOT].T
    return out



# revision 26
# speedup vs baseline: 1.0444x; 1.0444x over previous
"""DeeperGCN (GENConv softmax-aggr, 12 layers) on 8 Trainium2 NeuronCores.

Strategy: nodes are sharded across 8 cores by dst; each core owns its nodes'
incoming edges, packed dst-sorted into 128-edge chunks aligned to fixed
32-node PSUM windows. Per layer: all-gather bf16 node table -> dma_gather
src features + CCE-accumulate edge encodings -> relu/exp/mul ->
segment-softmax sums via TensorE matmuls (S.T @ [w|q]) into PSUM ->
nodewise MLP/LN in bf16 with fp32 residual state.

Softmax restructure (exact up to fp assoc):
  m = relu(y)+eps, logits = t*m; with shift-invariance and t>0:
  w_e = exp(t*relu(y)) ; den = sum w_e ; num = sum relu(y)*w_e
  agg = num/den + eps   (eps term exact to ~1e-16; empty segs give 0)
"""
import sys
import os
import numpy as np

if "/opt/trn_rl_repo" not in sys.path:
    sys.path.insert(0, "/opt/trn_rl_repo")

import ml_dtypes

# ---------------- problem constants (hardcoded per spec) ----------------
N = 20000
E = 640000
H = 128
L = 12
DIN = 3
DE = 4
DOUT = 16
EPS = 1e-7
LN_EPS = 1e-5
NCORES = 8
NLOC = N // NCORES          # 2500 nodes per core
BLK = 128                   # nodes per block (psum tile columns / partitions)
NBLK = (NLOC + BLK - 1) // BLK   # 20 blocks (last one partial)
NSLOT = NBLK * BLK          # 2560 node slots per core
WS = 32                     # psum window width (nodes per window)
NWIN = BLK // WS            # 4 windows per block
BPG = 1                     # blocks per dma_gather call

BF16 = ml_dtypes.bfloat16


# ======================================================================
# Host-side prep: shard + sort + pack the graph into per-core tensors
# ======================================================================
def _prep_graph(edge_index, edge_attr):
    src = np.asarray(edge_index[0]).astype(np.int64)
    dst = np.asarray(edge_index[1]).astype(np.int64)
    e_attr = np.asarray(edge_attr, dtype=np.float32)

    deg = np.bincount(dst, minlength=N)

    # --- node -> (core, block, col) slot assignment, degree balanced ---
    # LPT over blocks: equalizes per-block edge counts so the global max
    # window load (and thus c_win / e_pad) is minimal. Then LPT over the
    # 4 windows within each block.
    col_of = np.zeros(N, np.int32)
    blk_of = np.zeros(N, np.int32)
    for r in range(NCORES):
        nodes = np.arange(r * NLOC, (r + 1) * NLOC)
        order = nodes[np.argsort(-deg[nodes], kind="stable")]
        bload = np.zeros(NBLK, np.int64)
        bcnt = np.zeros(NBLK, np.int64)
        for nd in order:
            bsel = min(
                (bb for bb in range(NBLK) if bcnt[bb] < BLK),
                key=lambda bb: bload[bb],
            )
            blk_of[nd] = bsel
            bcnt[bsel] += 1
            bload[bsel] += deg[nd]
        for b in range(NBLK):
            bn = nodes[blk_of[nodes] == b]
            order = bn[np.argsort(-deg[bn], kind="stable")]
            wload = np.zeros(NWIN, np.int64)
            wcnt = np.zeros(NWIN, np.int64)
            for nd in order:
                j = min(
                    (jj for jj in range(NWIN) if wcnt[jj] < WS),
                    key=lambda jj: wload[jj],
                )
                col_of[nd] = j * WS + wcnt[j]
                wcnt[j] += 1
                wload[j] += deg[nd]
    # global table slot of each node (row in the all-gathered hn table)
    slot_of = (np.arange(N) // NLOC) * NSLOT + blk_of * BLK + col_of
    # table row under the chunked all-gather layout:
    # [group g=b//5][core r][b%5][col] so each 5-block group is gathered
    # by one collective into a contiguous table region
    own_of = np.arange(N) // NLOC
    tslot_of = ((blk_of // 5) * (NCORES * 5 * BLK) + own_of * (5 * BLK)
                + (blk_of % 5) * BLK + col_of)

    # --- chunk schedule: C chunks per block, round-robin over windows ---
    # window j of block b needs ceil(D_bj/128) chunks; C_WIN = global max
    own = dst // NLOC
    win_of_dst = col_of[dst] // WS
    blk_of_dst = blk_of[dst]
    # count edges per (core, block, window)
    key = (own * NBLK + blk_of_dst) * NWIN + win_of_dst
    wdeg = np.bincount(key, minlength=NCORES * NBLK * NWIN)
    c_win = int(np.max((wdeg + 127) // 128))
    C = NWIN * c_win
    e_pad = NBLK * C * 128
    assert e_pad % 16 == 0

    # sort edges by (core, block, window, col) so each window's edges are
    # contiguous and dst-col sorted
    sortkey = (key * BLK + col_of[dst]) * 1  # (core,blk,win,col)
    order = np.argsort(sortkey, kind="stable")
    s_src, s_dst = src[order], dst[order]
    s_attr = e_attr[order]

    gidx = np.zeros((NCORES, e_pad), np.int32)            # gather table rows
    attr_p = np.zeros((NCORES, e_pad, DE), np.float32)    # padded edge attrs
    scol = np.full((NCORES, e_pad), -1, np.int32)         # dst col in window (for S)

    # per (core, block, window): window j's chunks are consecutive
    # (c = j*c_win + q) so each window's psum group is sequential
    start = np.zeros(NCORES * NBLK * NWIN + 1, np.int64)
    np.cumsum(np.bincount(key[order], minlength=NCORES * NBLK * NWIN), out=start[1:])
    for r in range(NCORES):
        for b in range(NBLK):
            for j in range(NWIN):
                k = (r * NBLK + b) * NWIN + j
                e0, e1 = start[k], start[k + 1]
                nloc_e = e1 - e0
                # edge i of this window -> chunk q=i//128, slot s=i%128
                q, s = np.arange(nloc_e) // 128, np.arange(nloc_e) % 128
                epos = (b * C + j * c_win + q) * 128 + s
                gidx[r, epos] = tslot_of[s_src[e0:e1]]
                attr_p[r, epos] = s_attr[e0:e1]
                scol[r, epos] = col_of[s_dst[e0:e1]] - j * WS

    # --- build per-core device arrays ---
    # S: [128 slots, NBLK*C chunks, WS] one-hot dst-col matrices
    nchunks = NBLK * C
    S = np.zeros((NCORES, 128, nchunks, WS), np.float32)
    cc = (np.arange(e_pad) // 128)
    ss = (np.arange(e_pad) % 128)
    for r in range(NCORES):
        valid = scol[r] >= 0
        S[r, ss[valid], cc[valid], scol[r][valid]] = 1.0

    # idxs int16 [128, e_pad//16]: idx i at partition i%16, col i//16, x8
    idxs = np.zeros((NCORES, 128, e_pad // 16), np.int16)
    for r in range(NCORES):
        w = gidx[r].reshape(e_pad // 16, 16).T.astype(np.int16)  # [16, cols]
        idxs[r] = np.tile(w, (8, 1))

    # attr transposed [4, e_pad]
    attr_t = np.ascontiguousarray(attr_p.transpose(0, 2, 1))

    return dict(C=C, e_pad=e_pad, idxs=idxs, attr_t=attr_t,
                S=S.astype(BF16), slot_of=slot_of, col_of=col_of)


def _prep_inputs(inputs):
    g = _prep_graph(inputs["edge_index"], inputs["edge_attr"])
    x = np.asarray(inputs["x"], np.float32)
    # x_t [3, NSLOT] per core, node at its slot
    x_t = np.zeros((NCORES, DIN, NSLOT), np.float32)
    slot = g["slot_of"]
    for r in range(NCORES):
        nodes = np.arange(r * NLOC, (r + 1) * NLOC)
        x_t[r][:, slot[nodes] - r * NSLOT] = x[nodes].T

    f32 = np.float32
    w = {k: np.asarray(v, f32) for k, v in inputs.items()
         if k not in ("x", "edge_index", "edge_attr")}
    # triviality flags (structurally ones/zeros in setup_inputs)
    triv = (np.all(w["enc_node_b"] == 0) and np.all(w["enc_edge_b"] == 0)
            and np.all(w["mlp1_b"] == 0) and np.all(w["mlp2_b"] == 0)
            and np.all(w["mlp_ln_g"] == 1) and np.all(w["mlp_ln_b"] == 0)
            and np.all(w["ln_g"] == 1) and np.all(w["ln_b"] == 0)
            and np.all(w["lin_b"] == 0))
    assert triv, "non-trivial bias/LN params not supported by this kernel build"
    t_vals = [float(v) for v in np.asarray(inputs["t"], f32)]
    assert all(tv > 0 for tv in t_vals)

    w1_all = np.zeros((128, L * 2 * H), BF16)
    w2_all = np.zeros((128, L * 2 * H), BF16)
    for l in range(L):
        w1_all[:, l * 256:(l + 1) * 256] = w["mlp1_w"][l].astype(BF16)
        w2_all[:, (2 * l) * 128:(2 * l + 1) * 128] = w["mlp2_w"][l][0:128].astype(BF16)
        w2_all[:, (2 * l + 1) * 128:(2 * l + 2) * 128] = w["mlp2_w"][l][128:256].astype(BF16)

    per_core_common = dict(
        enc_node_w=np.asarray(w["enc_node_w"], f32),
        enc_edge_w=np.asarray(w["enc_edge_w"], f32),
        w1_all=w1_all, w2_all=w2_all,
        lin_w=np.asarray(w["lin_w"], f32).astype(BF16),
        ident=np.eye(128, dtype=BF16),
    )
    in_maps = []
    for r in range(NCORES):
        m = dict(per_core_common)
        m["x_t"] = x_t[r]
        m["attr_t"] = g["attr_t"][r]
        m["idxs_in"] = g["idxs"][r]
        m["s_in"] = g["S"][r]
        in_maps.append(m)
    return g, in_maps, t_vals


# ======================================================================
# Device program
# ======================================================================
DEBUG_TAPS = False


def _build_program(C, t_vals):
    import concourse.bass as bass
    import concourse.bacc as bacc
    import concourse.tile as tile
    import concourse.mybir as mybir

    dt = mybir.dt
    AF = mybir.ActivationFunctionType
    ALU = mybir.AluOpType
    c_win = C // NWIN
    e_pad = NBLK * C * 128
    nchunks = NBLK * C

    nc = bacc.Bacc("TRN2", target_bir_lowering=False, debug=False,
                   num_devices=NCORES, num_swdge_queues=4)

    def din(name, shape, dty):
        return nc.dram_tensor(name, shape, dty, kind="ExternalInput").ap()

    x_t = din("x_t", [DIN, NSLOT], dt.float32)
    attr_t = din("attr_t", [DE, e_pad], dt.float32)
    idxs_in = din("idxs_in", [128, e_pad // 16], dt.int16)
    s_in = din("s_in", [128, nchunks, WS], dt.bfloat16)
    enc_node_w = din("enc_node_w", [DIN, H], dt.float32)
    enc_edge_w = din("enc_edge_w", [DE, H], dt.float32)
    w1_in = din("w1_all", [128, L * 256], dt.bfloat16)
    w2_in = din("w2_all", [128, L * 256], dt.bfloat16)
    lin_in = din("lin_w", [128, DOUT], dt.bfloat16)
    ident_in = din("ident", [128, 128], dt.bfloat16)
    out_t = nc.dram_tensor("out_t", [DOUT, NSLOT], dt.float32,
                           kind="ExternalOutput").ap()
    dbg = {}
    if DEBUG_TAPS:
        e_pad_ = NBLK * C * 128
        for nm, shp, dty in [
            ("dbg_h0", [128, NSLOT], dt.float32),
            ("dbg_ea", [128, 1024], dt.bfloat16),
            ("dbg_tab", [NCORES * NSLOT, H], dt.bfloat16),
            ("dbg_y", [128, C * 128], dt.bfloat16),
            ("dbg_r", [128, C * 128], dt.bfloat16),
            ("dbg_wq", [128, C * 256], dt.bfloat16),
            ("dbg_dn", [128, 256], dt.float32),
            ("dbg_z", [128, H], dt.bfloat16),
            ("dbg_u", [128, 256], dt.bfloat16),
            ("dbg_h1", [128, NSLOT], dt.float32),
        ]:
            dbg[nm] = nc.dram_tensor(nm, shp, dty, kind="ExternalOutput").ap()

    # persistent SBUF
    sb = lambda name, shape, dty: nc.alloc_sbuf_tensor(name, list(shape), dty).ap()
    x_sb = sb("x_sb", [DIN, NSLOT], dt.float32)
    idxs_sb = sb("idxs_sb", [128, e_pad // 16], dt.int16)
    s_sb = sb("s_sb", [128, nchunks, WS], dt.bfloat16)
    encn_sb = sb("encn_sb", [DIN, H], dt.float32)
    ence_sb = sb("ence_sb", [DE, H], dt.float32)
    w1_sb = sb("w1_sb", [128, L * 256], dt.bfloat16)
    w2_sb = sb("w2_sb", [128, L * 256], dt.bfloat16)
    lin_sb = sb("lin_sb", [128, DOUT], dt.bfloat16)
    ident_sb = sb("ident_sb", [128, 128], dt.bfloat16)
    h_sb = sb("h_sb", [128, NSLOT], dt.float32)      # [node_in_blk, blk*H+ch] fp32 state
    eps_sb = sb("eps_sb", [128, 1], dt.float32)      # LN_EPS per-partition const
    hn_sb = sb("hn_sb", [128, NSLOT], dt.bfloat16)   # relu(LN(h)) per block
    zero_sb = sb("zero_sb", [128, C * 128], dt.bfloat16)  # relu via tt-max

    RG = [list(range(NCORES))]

    with tile.TileContext(nc) as tc:
        with (
            tc.tile_pool(name="dram", bufs=1, space="DRAM") as dram,
            tc.tile_pool(name="ypool", bufs=4) as ypool,
            tc.tile_pool(name="eapool", bufs=2) as eapool,
            tc.tile_pool(name="wqpool", bufs=2) as wqpool,
            tc.tile_pool(name="nsmall", bufs=3) as nsmall,
            tc.tile_pool(name="nwork", bufs=3) as nwork,
            tc.tile_pool(name="dn_ps", bufs=2, space="PSUM") as dn_ps_pool,
            tc.tile_pool(name="mlp_ps", bufs=2, space="PSUM") as mlp_ps_pool,
            tc.tile_pool(name="tp_ps", bufs=2, space="PSUM") as tp_ps_pool,
            tc.tile_pool(name="setup", bufs=3) as setup_pool,
        ):
            ea_d = dram.tile([128, e_pad], dt.bfloat16)
            ag_in = dram.tile([NSLOT, H], dt.bfloat16)
            # double-buffered node table: layer l gathers from tab[l%2],
            # its collectives write tab[(l+1)%2] (no WAR with own gathers)
            tab = [dram.tile([NCORES * NSLOT, H], dt.bfloat16, name=f"tab{i}")
                   for i in range(2)]

            # ---------------- load constants ----------------
            nc.sync.dma_start(x_sb, x_t)
            nc.sync.dma_start(idxs_sb, idxs_in)
            nc.sync.dma_start(s_sb, s_in)
            nc.sync.dma_start(encn_sb, enc_node_w)
            nc.sync.dma_start(ence_sb, enc_edge_w)
            nc.sync.dma_start(w1_sb, w1_in)
            nc.sync.dma_start(w2_sb, w2_in)
            nc.sync.dma_start(lin_sb, lin_in)
            nc.sync.dma_start(ident_sb, ident_in)
            nc.gpsimd.memset(eps_sb, LN_EPS)
            nc.gpsimd.memset(zero_sb, 0.0)

            # ---------------- encoders ----------------
            for b in range(NBLK):
                h_ps = tp_ps_pool.tile([128, H], dt.float32, tag="tp")
                nc.tensor.matmul(h_ps, lhsT=x_sb[:, b * 128:(b + 1) * 128],
                                 rhs=encn_sb, start=True, stop=True)
                nc.vector.tensor_copy(h_sb[:, b * H:(b + 1) * H], h_ps)
                nc.scalar.copy(hn_sb[:, b * H:(b + 1) * H], h_ps)
                nc.sync.dma_start(ag_in[b * 128:(b + 1) * 128, :],
                                  hn_sb[:, b * H:(b + 1) * H])
            for g4 in range(nchunks // 4):
                atile = setup_pool.tile([DE, 512], dt.float32, tag="attr")
                nc.sync.dma_start(atile, attr_t[:, g4 * 512:(g4 + 1) * 512])
                ea_ps = mlp_ps_pool.tile([128, 512], dt.float32, tag="mlp")
                for k in range(4):
                    nc.tensor.matmul(ea_ps[:, k * 128:(k + 1) * 128],
                                     lhsT=atile[:, k * 128:(k + 1) * 128],
                                     rhs=ence_sb, start=True, stop=True)
                ea_sb = setup_pool.tile([128, 512], dt.bfloat16, tag="east")
                if g4 % 2 == 0:
                    nc.vector.tensor_copy(ea_sb, ea_ps)
                else:
                    nc.scalar.copy(ea_sb, ea_ps)
                nc.sync.dma_start(ea_d[:, g4 * 512:(g4 + 1) * 512], ea_sb)
            for g4c in range(4):
                nc.gpsimd.collective_compute(
                    "AllGather", ALU.bypass, replica_groups=RG,
                    ins=[ag_in[g4c * 640:(g4c + 1) * 640, :]],
                    outs=[tab[0][g4c * 5120:(g4c + 1) * 5120, :]])
            if DEBUG_TAPS:
                nc.sync.dma_start(dbg["dbg_h0"], h_sb)
                nc.sync.dma_start(dbg["dbg_ea"], ea_d[:, 0:1024])
                nc.sync.dma_start(dbg["dbg_tab"], tab[0][:, :])

            # ---------------- layers ----------------
            for l in range(L):
                y = None
                for b in range(NBLK):
                    ni = C * 128
                    y = ypool.tile([128, C, 128], dt.bfloat16, tag="y")
                    half = (C * 128) // 2
                    for hf in range(2):
                        nc.gpsimd.dma_gather(
                            y[:, hf * (C // 2):(hf + 1) * (C // 2), :],
                            tab[l % 2][:, :],
                            idxs_sb[:, (b * C * 128 + hf * half) // 16:
                                    (b * C * 128 + (hf + 1) * half) // 16],
                            num_idxs=half, num_idxs_reg=half, elem_size=H,
                            single_packet=False, queue_num=(2 * b + hf) % 4)
                    eat = eapool.tile([128, C * 128], dt.bfloat16, tag="ea")
                    nc.sync.dma_start(eat, ea_d[:, b * C * 128:(b + 1) * C * 128])
                    yb = y[:, :, :]
                    # y += ea ; r = relu(y)  (both in place)
                    nc.vector.tensor_tensor(y.opt(), y.opt(), eat, ALU.add)
                    if DEBUG_TAPS and l == 0 and b == 0:
                        nc.sync.dma_start(dbg["dbg_y"], y.opt())
                    nc.vector.tensor_tensor(y.opt(), y.opt(), zero_sb, ALU.max)
                    if DEBUG_TAPS and l == 0 and b == 0:
                        nc.sync.dma_start(dbg["dbg_r"], y.opt())
                    wq = wqpool.tile([128, C, 256], dt.bfloat16, tag="wq")
                    nc.scalar.activation(wq[:, :, 0:128], yb, AF.Exp,
                                         scale=t_vals[l])
                    nc.vector.tensor_tensor(wq[:, :, 128:256], yb,
                                            wq[:, :, 0:128], ALU.mult)
                    if DEBUG_TAPS and l == 0 and b == 0:
                        nc.sync.dma_start(dbg["dbg_wq"], wq.opt())
                    dn = dn_ps_pool.tile([128, 256], dt.float32, tag="dn")
                    for c in range(C):
                        j, q = c // c_win, c % c_win
                        nc.tensor.matmul(
                            dn[32 * j:32 * j + 32, :],
                            lhsT=s_sb[:, b * C + c, :], rhs=wq[:, c, :],
                            start=(q == 0), stop=(q == c_win - 1),
                            tile_position=(0, 32 * j))
                    _nodewise(nc, tc, l, b, dn,
                              h_sb, hn_sb, w1_sb, w2_sb, lin_sb, ident_sb,
                              eps_sb, nsmall, nwork, mlp_ps_pool, tp_ps_pool,
                              ag_in, out_t, dt, AF, ALU, dbg)
                    # deferred chunk collectives: group g (blocks 5g..5g+4)
                    # fires at block 5g+8 so the ag_in writes are already
                    # done when the Pool queue reaches the trigger
                    if l < L - 1 and b >= 8 and (b - 8) % 5 == 0:
                        g4c = (b - 8) // 5
                        nc.gpsimd.collective_compute(
                            "AllGather", ALU.bypass, replica_groups=RG,
                            ins=[ag_in[g4c * 640:(g4c + 1) * 640, :]],
                            outs=[tab[(l + 1) % 2][g4c * 5120:
                                                   (g4c + 1) * 5120, :]])
                if l < L - 1:
                    nc.gpsimd.collective_compute(
                        "AllGather", ALU.bypass, replica_groups=RG,
                        ins=[ag_in[3 * 640:4 * 640, :]],
                        outs=[tab[(l + 1) % 2][3 * 5120:4 * 5120, :]])

    nc.compile()
    return nc


def _nodewise(nc, tc, l, b, dn, h_sb, hn_sb, w1_sb, w2_sb, lin_sb, ident_sb,
              eps_sb, nsmall, nwork, mlp_ps_pool, tp_ps_pool, ag_in, out_t,
              dt, AF, ALU, dbg):
    H_ = H
    hsl = slice(b * H_, (b + 1) * H_)
    # agg = num/(den+1e-16);  z = agg + hn   [node, ch]
    tmp = nwork.tile([128, H_], dt.float32, tag="tmp")
    nc.vector.tensor_scalar_add(tmp, dn[:, 0:128], 1e-16)
    rec = nwork.tile([128, H_], dt.float32, tag="rec")
    nc.vector.reciprocal_approx_fast(rec, tmp)
    agg = nwork.tile([128, H_], dt.float32, tag="agg")
    nc.vector.tensor_tensor(agg, dn[:, 128:256], rec, ALU.mult)
    z = nwork.tile([128, H_], dt.bfloat16, tag="z")
    nc.vector.tensor_tensor(z, agg, hn_sb[:, hsl], ALU.add)
    if dbg and l == 0 and b == 0:
        dnc = nwork.tile([128, 256], dt.float32, tag="dbgdn")
        nc.vector.tensor_copy(dnc, dn)
        nc.sync.dma_start(dbg["dbg_dn"], dnc)
        nc.sync.dma_start(dbg["dbg_z"], z)
    # zT for mm1
    zT_ps = tp_ps_pool.tile([128, H_], dt.bfloat16, tag="tp")
    nc.tensor.transpose(zT_ps, z, ident_sb)
    zT = nwork.tile([128, H_], dt.bfloat16, tag="zT")
    nc.vector.tensor_copy(zT, zT_ps)
    # mm1: out1[n, 2H] = z @ w1
    out1 = mlp_ps_pool.tile([128, 256], dt.float32, tag="mlp")
    nc.tensor.matmul(out1, lhsT=zT, rhs=w1_sb[:, l * 256:(l + 1) * 256],
                     start=True, stop=True)
    # LN(out1) -> relu -> u  (g=1, b=0)
    st6 = nsmall.tile([128, 6], dt.float32, tag="st6")
    nc.vector.bn_stats(st6, out1)
    mv = nsmall.tile([128, 2], dt.float32, tag="mv")
    nc.vector.bn_aggr(mv, st6)
    sq = nsmall.tile([128, 1], dt.float32, tag="sq")
    nc.scalar.activation(sq, mv[:, 1:2], AF.Sqrt, bias=eps_sb[:, 0:1])
    rstd = nsmall.tile([128, 1], dt.float32, tag="rstd")
    nc.vector.reciprocal(rstd, sq)
    nmr = nsmall.tile([128, 1], dt.float32, tag="nmr")
    nc.vector.tensor_scalar(nmr, mv[:, 0:1], rstd[:, 0:1], -1.0,
                            ALU.mult, ALU.mult)
    u = nwork.tile([128, 256], dt.bfloat16, tag="u")
    nc.scalar.activation(u, out1, AF.Relu, bias=nmr[:, 0:1],
                         scale=rstd[:, 0:1])
    if dbg and l == 0 and b == 0:
        nc.sync.dma_start(dbg["dbg_u"], u)
    # uT (two halves) for mm2
    uT_ps = tp_ps_pool.tile([128, 256], dt.bfloat16, tag="tp")
    nc.tensor.transpose(uT_ps[:, 0:128], u[:, 0:128], ident_sb)
    nc.tensor.transpose(uT_ps[:, 128:256], u[:, 128:256], ident_sb)
    uT = nwork.tile([128, 256], dt.bfloat16, tag="uT")
    nc.vector.tensor_copy(uT, uT_ps)
    # mm2: dh[n, ch] = u @ w2
    out2 = mlp_ps_pool.tile([128, 128], dt.float32, tag="mlp")
    nc.tensor.matmul(out2, lhsT=uT[:, 0:128],
                     rhs=w2_sb[:, (2 * l) * 128:(2 * l + 1) * 128],
                     start=True, stop=False)
    nc.tensor.matmul(out2, lhsT=uT[:, 128:256],
                     rhs=w2_sb[:, (2 * l + 1) * 128:(2 * l + 2) * 128],
                     start=False, stop=True)
    # residual (layer 0: h = mlp out only)
    if l == 0:
        nc.vector.tensor_copy(h_sb[:, hsl], out2)
    else:
        nc.vector.tensor_tensor(h_sb[:, hsl], h_sb[:, hsl], out2, ALU.add)
    # prenorm -> hn (next layer's table; layer L-1 computes head input w/ g0)
    st6b = nsmall.tile([128, 6], dt.float32, tag="st6")
    nc.vector.bn_stats(st6b, h_sb[:, hsl])
    mvb = nsmall.tile([128, 2], dt.float32, tag="mv")
    nc.vector.bn_aggr(mvb, st6b)
    sqb = nsmall.tile([128, 1], dt.float32, tag="sq")
    nc.scalar.activation(sqb, mvb[:, 1:2], AF.Sqrt, bias=eps_sb[:, 0:1])
    rstdb = nsmall.tile([128, 1], dt.float32, tag="rstd")
    nc.vector.reciprocal(rstdb, sqb)
    nmrb = nsmall.tile([128, 1], dt.float32, tag="nmr")
    nc.vector.tensor_scalar(nmrb, mvb[:, 0:1], rstdb[:, 0:1], -1.0,
                            ALU.mult, ALU.mult)
    nc.scalar.activation(hn_sb[:, hsl], h_sb[:, hsl], AF.Relu,
                         bias=nmrb[:, 0:1], scale=rstdb[:, 0:1])
    if dbg and l == 0 and b == NBLK - 1:
        nc.sync.dma_start(dbg["dbg_h1"], h_sb)
    if l < L - 1:
        nc.sync.dma_start(ag_in[b * 128:(b + 1) * 128, :], hn_sb[:, hsl])
    else:
        # final head: out = relu(LN(h,g0)) @ lin_w
        hT_ps = tp_ps_pool.tile([128, H_], dt.bfloat16, tag="tp")
        nc.tensor.transpose(hT_ps, hn_sb[:, hsl], ident_sb)
        hT = nwork.tile([128, H_], dt.bfloat16, tag="zT")
        nc.vector.tensor_copy(hT, hT_ps)
        outf_ps = mlp_ps_pool.tile([DOUT, 128], dt.float32, tag="mlp")
        nc.tensor.matmul(outf_ps, lhsT=lin_sb, rhs=hT, start=True, stop=True)
        outf = nwork.tile([DOUT, 128], dt.float32, tag="outf")
        nc.vector.tensor_copy(outf, outf_ps)
        nc.sync.dma_start(out_t[:, b * 128:(b + 1) * 128], outf)


# ======================================================================
# Entry point
# ======================================================================
_CACHE = {}


def kernel(**inputs):
    g, in_maps, t_vals = _prep_inputs(inputs)
    C = g["C"]
    key = (C, tuple(t_vals))
    if key not in _CACHE:
        _CACHE[key] = _build_program(C, t_vals)
    nc = _CACHE[key]

    from concourse import bass_utils
    res = bass_utils.run_bass_kernel_spmd(
        nc, in_maps, core_ids=list(range(NCORES)))

    out = np.zeros((N, DOUT), np.float32)
    slot = g["slot_of"]
    for r in range(NCORES):
        nodes = np.arange(r * NLOC, (r + 1) * NLOC)
        out[nodes] = res.results[r]["out_t"][:, slot[nodes] - r * NSLOT].T
    return out

